# revision 11
# baseline (speedup 1.0000x reference)
"""Trainium2 Bass kernel for nn_DiffModule_40827959116531 (sparse_attention).

Algebraic restructure of the reference (per batch element b), exploiting
softmax shift-invariance and weight folding (host-precomputed products):

    M = W1 @ W2.T            V = W1 @ (W3a_top + W3a_bot)     U = W1 @ W3a_bot
    sm    = src @ M                               # (L, D)   fp8 DR
    score = (sm @ trg.T + row_n) / sqrt(O)        # (L, N)   fp8 DR
      (row_n = trg @ (W2 @ b1) + b1@b2; the per-l term sv@b2 is dropped -
       softmax over n is invariant to it)
    e     = exp(score)                            #          exp on ACT -> fp8
    tvp   = trg @ U                               # (N, O)   fp8 DR
    ctxr  = e @ tvp                               # (L, O)   fp8 DR, unnormalized
    sv3   = src @ V                               # (L, O)   bf16
    h2    = relu(sv3 - ctxr/sum_n(e) + r3)        # r3 = b3a + b1 @ W3a_top
    out   = h2 @ W3b + b3b                        # (L, O)   bf16

6 full-size matmul units instead of the reference's 8; the 4 attention-path
units run as fp8e4 DoubleRow (2 K-subtiles per instruction, ~2x bf16 rate).
fp8 is safe on this path: scores have std ~0.33 so softmax is near-uniform
and ctx has std ~0.008 vs h's 0.195 - quantization errors there are
attenuated ~25x. sv3/fc3b pass errors straight through, so they stay bf16.
The softmax denominator is summed off-PE (DVE slab adds + gpsimd partition
all-reduce) and folded into the DVE combine, so the PE runs nothing but the
2048 full-size matmuls. Measured ~470 us vs the 960 us all-bf16 8-unit
baseline (PE busy ~444 us, within ~2% of the instruction-stream floor).

Sharding: data-parallel over B=32 across 8 cores (4 batch elems per core);
weights replicated. Host marshals activations to [D, tokens] layouts and
quantizes with fixed power-of-2 scales (inputs have std 1; >2.8x headroom
to the fp8e4 max of 240 everywhere).
"""

import math
from contextlib import ExitStack

import ml_dtypes
import numpy as np

import concourse.bass as bass
import concourse.mybir as mybir
import concourse.tile as tile
from concourse import bacc
from concourse.bass_isa import ReduceOp
from concourse.bass_utils import run_bass_kernel_spmd

P = 128
B_FULL = 32
N_CORES = 8
BS = B_FULL // N_CORES  # 4 batch elements per core
L = 1024
N = 1024
D = 1024
O = 1024

F32 = mybir.dt.float32
BF16 = mybir.dt.bfloat16
F8 = mybir.dt.float8e4
AF = mybir.ActivationFunctionType
ALU = mybir.AluOpType
DR = mybir.MatmulPerfMode.DoubleRow
NP_BF16 = ml_dtypes.bfloat16
NP_F8 = ml_dtypes.float8_e4m3

LC = 512          # l-chunk size for phase B
N_LC = L // LC    # 2 chunks
KD = D // P       # 8 contraction tiles over D
KO = O // P       # 8 tiles over O
KN = N // P       # 8 tiles over N
KH = KD // 2      # 4 DoubleRow steps over D

# fixed power-of-2 quantization scales (inputs are N(0,1); margins >= 2.8x)
S_SRC = 16.0
S_TRG = 16.0
S_M = 2048.0
S_SM = 32.0
S_E = 16.0
S_U = 4096.0
S_TVP = 64.0

CS_SM = S_SM / (S_SRC * S_M)                  # sm psum -> fp8 drain scale
CS_E = 1.0 / (S_SM * S_TRG * math.sqrt(O))    # score psum -> exp input scale
CS_TVP = S_TVP / (S_TRG * S_U)                # tvp psum -> fp8 drain scale
C_CTX = -1.0 / S_TVP                          # ctx*rbc -> true-value scale


def _load_weight(nc, dst, w_dram, ktiles):
    # one 3D-AP DMA: dram (ktiles*128, X) rows -> sbuf [128, ktiles, X]
    nc.sync.dma_start(
        dst[:, :, :], w_dram.ap().rearrange("(kk p) d -> p kk d", p=P))


def _load_st(nc, dest, dram, b):
    nc.sync.dma_start(
        dest[:, :, :], dram.ap()[b].rearrange("(kk p) d -> p kk d", p=P))


def _build(nc, tc):
    src8_d = nc.dram_tensor("srcT8", [BS, D, L], F8, kind="ExternalInput")
    trg8_d = nc.dram_tensor("trgT8", [BS, D, N], F8, kind="ExternalInput")
    srcbf_d = nc.dram_tensor("srcTbf", [BS, D, L], BF16, kind="ExternalInput")
    m8_d = nc.dram_tensor("M8", [D, D], F8, kind="ExternalInput")
    u8_d = nc.dram_tensor("U8", [D, O], F8, kind="ExternalInput")
    vbf_d = nc.dram_tensor("Vbf", [D, O], BF16, kind="ExternalInput")
    w3b_d = nc.dram_tensor("W3bbf", [O, O], BF16, kind="ExternalInput")
    r3_d = nc.dram_tensor("r3", [O], F32, kind="ExternalInput")
    b3b_d = nc.dram_tensor("b3bf32", [O], F32, kind="ExternalInput")
    ebias_d = nc.dram_tensor("ebias", [BS, N], F32, kind="ExternalInput")
    out = nc.dram_tensor("out", [BS, L, O], F32, kind="ExternalOutput")

    ctx = ExitStack()
    singles = ctx.enter_context(tc.tile_pool(name="singles", bufs=1))
    stp = ctx.enter_context(tc.tile_pool(name="stp", bufs=2))
    actp = ctx.enter_context(tc.tile_pool(name="actp", bufs=1))
    lcp = ctx.enter_context(tc.tile_pool(name="lcp", bufs=1))
    smallp = ctx.enter_context(tc.tile_pool(name="smallp", bufs=2))
    hlp = ctx.enter_context(tc.tile_pool(name="hlp", bufs=3))
    outp = ctx.enter_context(tc.tile_pool(name="outp", bufs=3))
    psum = ctx.enter_context(tc.tile_pool(name="psum", bufs=8, space="PSUM"))

    # ---- constants ----
    m8 = singles.tile([P, KD, D], F8)
    u8 = singles.tile([P, KD, O], F8)
    vbf = singles.tile([P, KD, O], BF16)
    w3b = singles.tile([P, KO, O], BF16)
    r3col = singles.tile([P, KO], F32)
    b3bfull = singles.tile([P, O], F32)
    # batch-0 activations + the weights needed first: the very first matmul
    # group (sm, j=0) reads only m8/s8 k-slabs 0-1, so those DMAs go first.
    s8_0 = stp.tile([P, KD, L], F8, tag="st")
    t8_0 = stp.tile([P, KD, N], F8, tag="st")
    sbf_0 = stp.tile([P, KD, L], BF16, tag="st")
    eb0 = smallp.tile([P, KN], F32, tag="ebias")
    # 4KB warmups absorb the cold-queue ramp ahead of the critical slabs
    nc.sync.dma_start(r3col[:], r3_d.ap().rearrange("(oo op) -> op oo", op=P))
    nc.scalar.dma_start(
        eb0[:], ebias_d.ap()[0].rearrange("(nn np) -> np nn", np=P))
    # staged loads: the first sm matmul gates only on the first k-slab pair
    # of m8/s8; 3 DMAs/queue stay within the 4-deep queue-semaphore rotation
    def _stage(dst, dram_ap):
        for lo, hi in ((0, 2), (2, 4), (4, KD)):
            yield dst[:, lo:hi, :], dram_ap[lo * P:hi * P, :].rearrange(
                "(kk p) d -> p kk d", p=P)
    for dst_ap, src_ap in _stage(m8, m8_d.ap()):
        nc.sync.dma_start(dst_ap, src_ap)
    for dst_ap, src_ap in _stage(s8_0, src8_d.ap()[0]):
        nc.scalar.dma_start(dst_ap, src_ap)
    nc.sync.dma_start(
        u8[:, :, :], u8_d.ap().rearrange("(kk p) d -> p kk d", p=P))
    nc.scalar.dma_start(
        t8_0[:, :, :], trg8_d.ap()[0].rearrange("(kk p) d -> p kk d", p=P))
    nc.sync.dma_start(
        b3bfull[:], bass.AP(tensor=b3b_d.ap().tensor, offset=0, ap=[[0, P], [1, O]]))
    nc.scalar.dma_start(
        sbf_0[:, :, :], srcbf_d.ap()[0].rearrange("(kk p) d -> p kk d", p=P))
    _load_weight(nc, vbf, vbf_d, KD)
    _load_weight(nc, w3b, w3b_d, KO)

    for b in range(BS):
        if b == 0:
            s8, t8, sbf, ebcol = s8_0, t8_0, sbf_0, eb0
        else:
            s8 = stp.tile([P, KD, L], F8, tag="st")
            t8 = stp.tile([P, KD, N], F8, tag="st")
            sbf = stp.tile([P, KD, L], BF16, tag="st")
            ebcol = smallp.tile([P, KN], F32, tag="ebias")
            # prefetches ride the scalar hwdge queue (idle after batch 0) so
            # the sync queue carries only output stores - a 2MB prefetch on
            # the store queue would backpressure the 3-deep outp pool
            nc.scalar.dma_start(
                ebcol[:], ebias_d.ap()[b].rearrange("(nn np) -> np nn", np=P))
            for dram, dest in ((src8_d, s8), (trg8_d, t8), (srcbf_d, sbf)):
                nc.scalar.dma_start(
                    dest[:, :, :],
                    dram.ap()[b].rearrange("(kk p) d -> p kk d", p=P))

        # ---- phase A: sm = src@M and tvp = trg@U, both fp8 DoubleRow ----
        sm8 = actp.tile([P, KD, L], F8, tag="sm8")
        tvp8 = actp.tile([P, KN, O], F8, tag="tvp8")
        for j in range(KD):
            for lc in range(N_LC):
                ps = psum.tile([P, LC], F32)
                for kk in range(KH):
                    nc.tensor.matmul(
                        ps[:], m8[:, 2 * kk:2 * kk + 2, j * P:(j + 1) * P],
                        s8[:, 2 * kk:2 * kk + 2, lc * LC:(lc + 1) * LC],
                        start=(kk == 0), stop=(kk == KH - 1), perf_mode=DR)
                nc.scalar.activation(
                    sm8[:, j, lc * LC:(lc + 1) * LC], ps[:], AF.Identity,
                    scale=CS_SM)
        for i in range(KN):
            for oc in range(O // LC):
                ps = psum.tile([P, LC], F32)
                for kk in range(KH):
                    nc.tensor.matmul(
                        ps[:], t8[:, 2 * kk:2 * kk + 2, i * P:(i + 1) * P],
                        u8[:, 2 * kk:2 * kk + 2, oc * LC:(oc + 1) * LC],
                        start=(kk == 0), stop=(kk == KH - 1), perf_mode=DR)
                nc.scalar.activation(
                    tvp8[:, i, oc * LC:(oc + 1) * LC], ps[:], AF.Identity,
                    scale=CS_TVP)

        # ---- phase B: per l-chunk ----
        for lc in range(N_LC):
            lsl = slice(lc * LC, (lc + 1) * LC)
            # scoreT [n-part, l] fp8 DR; e8 = exp(score + row_n/sqrt(O)) * S_E
            e8 = lcp.tile([P, KN, LC], F8, tag="e8")
            for i in range(KN):
                ps = psum.tile([P, LC], F32)
                for kk in range(KH):
                    nc.tensor.matmul(
                        ps[:], t8[:, 2 * kk:2 * kk + 2, i * P:(i + 1) * P],
                        sm8[:, 2 * kk:2 * kk + 2, lsl],
                        start=(kk == 0), stop=(kk == KH - 1), perf_mode=DR)
                nc.scalar.activation(e8[:, i, :], ps[:], AF.Exp,
                                     scale=CS_E, bias=ebcol[:, i:i + 1])
            # sv3 (bf16) j=0..1 gives the PE work while ACT drains e8
            sv3s = lcp.tile([P, KO, LC], BF16, tag="sv3s")

            def sv3_group(j):
                ps = psum.tile([P, LC], F32)
                for k in range(KD):
                    nc.tensor.matmul(
                        ps[:], vbf[:, k, j * P:(j + 1) * P], sbf[:, k, lsl],
                        start=(k == 0), stop=(k == KD - 1))
                nc.scalar.activation(sv3s[:, j, :], ps[:], AF.Identity,
                                     bias=r3col[:, j:j + 1])

            for j in range(2):
                sv3_group(j)
            # denominator: DVE-sum the 8 n-tile slabs, gpsimd-reduce across
            # partitions (PE stays out of the softmax normalization entirely)
            e_sum = smallp.tile([P, LC], F32, tag="esum", bufs=1)
            nc.vector.tensor_add(e_sum[:], e8[:, 0, :], e8[:, 1, :])
            for i in range(2, KN):
                nc.vector.tensor_add(e_sum[:], e_sum[:], e8[:, i, :])
            d_bc = smallp.tile([P, LC], F32, tag="dbc", bufs=1)
            nc.gpsimd.partition_all_reduce(d_bc[:], e_sum[:], P, ReduceOp.add)
            for j in range(2, 4):
                sv3_group(j)
            rbc = smallp.tile([P, LC], F32, tag="rbc", bufs=1)
            nc.vector.reciprocal(rbc[:], d_bc[:])
            for j in range(4, KO):
                sv3_group(j)
            # ctx_raw (fp8 DR on unnormalized e8); normalization folds into
            # the DVE combine: h = sv3 - (ctx_raw*rbc)/S_TVP + r3
            h2 = lcp.tile([P, KO, LC], BF16, tag="h2")
            for j in range(KO):
                ps = psum.tile([P, LC], F32)
                for i in range(KH):
                    nc.tensor.matmul(
                        ps[:], tvp8[:, 2 * i:2 * i + 2, j * P:(j + 1) * P],
                        e8[:, 2 * i:2 * i + 2, :],
                        start=(i == 0), stop=(i == KH - 1), perf_mode=DR)
                ctxn = hlp.tile([P, LC], F32, tag="ctxn")
                nc.vector.tensor_mul(ctxn[:], ps[:], rbc[:])
                hl = hlp.tile([P, LC], BF16, tag="hl")
                nc.vector.scalar_tensor_tensor(
                    hl[:], ctxn[:], C_CTX, sv3s[:, j, :],
                    op0=ALU.mult, op1=ALU.add)
                nc.scalar.activation(h2[:, j, :], hl[:], AF.Relu)
            # fc3b: out natural [l-part, o] bf16, + b3b; one 1KB-row DMA per lt.
            # For the very last chunk, stores split per-oc (and the final oc
            # per-half) across the sync+scalar queues - both idle by then -
            # so the post-compute drain shrinks from ~4us to ~1.5us.
            last_chunk = (b == BS - 1 and lc == N_LC - 1)
            for lt in range(LC // P):
                o_sb = outp.tile([P, O], F32, tag="osb")
                row0 = lc * LC + lt * P
                for oc in range(O // LC):
                    ps = psum.tile([P, LC], F32)
                    for k in range(KO):
                        nc.tensor.matmul(
                            ps[:], h2[:, k, lt * P:(lt + 1) * P],
                            w3b[:, k, oc * LC:(oc + 1) * LC],
                            start=(k == 0), stop=(k == KO - 1))
                    final_oc = last_chunk and lt == LC // P - 1 and oc == O // LC - 1
                    if final_oc:
                        # split the critical last add+store: 2 half adds,
                        # 4 quarter stores alternating queues
                        for hh in range(2):
                            sl = slice(oc * LC + hh * (LC // 2),
                                       oc * LC + (hh + 1) * (LC // 2))
                            psl = slice(hh * (LC // 2), (hh + 1) * (LC // 2))
                            nc.vector.tensor_add(o_sb[:, sl], ps[:, psl],
                                                 b3bfull[:, sl])
                            for qq in range(2):
                                qsl = slice(oc * LC + (2 * hh + qq) * (LC // 4),
                                            oc * LC + (2 * hh + qq + 1) * (LC // 4))
                                q = nc.sync if qq == 0 else nc.scalar
                                q.dma_start(
                                    out.ap()[b, row0:row0 + P, qsl],
                                    o_sb[:, qsl])
                    else:
                        nc.vector.tensor_add(o_sb[:, oc * LC:(oc + 1) * LC],
                                             ps[:],
                                             b3bfull[:, oc * LC:(oc + 1) * LC])
                        if last_chunk:
                            q = nc.sync if (lt + oc) % 2 == 0 else nc.scalar
                            q.dma_start(
                                out.ap()[b, row0:row0 + P,
                                         oc * LC:(oc + 1) * LC],
                                o_sb[:, oc * LC:(oc + 1) * LC])
                if not last_chunk:
                    nc.sync.dma_start(out.ap()[b, row0:row0 + P, :], o_sb[:])

    ctx.close()


_NC_CACHE = None


def _get_nc():
    global _NC_CACHE
    if _NC_CACHE is None:
        nc = bacc.Bacc("TRN2", target_bir_lowering=False, debug=False,
                       num_devices=N_CORES)
        with tile.TileContext(nc) as tc:
            _build(nc, tc)
        nc.compile()
        _NC_CACHE = nc
    return _NC_CACHE


def _q8(x, scale):
    return np.clip(x * scale, -240.0, 240.0).astype(NP_F8)


def kernel(**inputs):
    nc = _get_nc()
    src = np.asarray(inputs["src"], dtype=np.float32)
    trg = np.asarray(inputs["trg"], dtype=np.float32)
    W1 = np.asarray(inputs["W1"], np.float32)
    W2 = np.asarray(inputs["W2"], np.float32)
    W3a = np.asarray(inputs["W3a"], np.float32)
    W3b = np.asarray(inputs["W3b"], np.float32)
    b1 = np.asarray(inputs["b1"], np.float32)
    b2 = np.asarray(inputs["b2"], np.float32)
    b3a = np.asarray(inputs["b3a"], np.float32)
    b3b = np.asarray(inputs["b3b"], np.float32)

    # host-side weight folding (O(D^3) once, vs O(B*L*D^2) on device)
    W3a_top, W3a_bot = W3a[:O], W3a[O:]
    M = W1 @ W2.T
    U = W1 @ W3a_bot
    V = W1 @ (W3a_top + W3a_bot)
    r3 = b3a + b1 @ W3a_top
    ebias = (trg @ (W2 @ b1) + float(b1 @ b2)) / math.sqrt(O) + math.log(S_E)

    src_t = np.ascontiguousarray(src.transpose(0, 2, 1))   # (B, D, L)
    trg_t = np.ascontiguousarray(trg.transpose(0, 2, 1))   # (B, D, N)
    shared = {
        "M8": _q8(M, S_M),
        "U8": _q8(U, S_U),
        "Vbf": V.astype(NP_BF16),
        "W3bbf": W3b.astype(NP_BF16),
        "r3": np.ascontiguousarray(r3),
        "b3bf32": np.ascontiguousarray(b3b),
    }
    src8 = _q8(src_t, S_SRC)
    trg8 = _q8(trg_t, S_TRG)
    srcbf = src_t.astype(NP_BF16)
    in_maps = []
    for c in range(N_CORES):
        m = dict(shared)
        m["srcT8"] = src8[c * BS:(c + 1) * BS]
        m["trgT8"] = trg8[c * BS:(c + 1) * BS]
        m["srcTbf"] = srcbf[c * BS:(c + 1) * BS]
        m["ebias"] = np.ascontiguousarray(ebias[c * BS:(c + 1) * BS])
        in_maps.append(m)
    res = run_bass_kernel_spmd(nc, in_maps, core_ids=list(range(N_CORES)))
    return np.concatenate([r["out"] for r in res.results], axis=0)



# revision 15
# speedup vs baseline: 1.0092x; 1.0092x over previous
"""Trainium2 Bass kernel for nn_DiffModule_40827959116531 (sparse_attention).

Algebraic restructure of the reference (per batch element b), exploiting
softmax shift-invariance and weight folding (host-precomputed products):

    M = W1 @ W2.T            V = W1 @ (W3a_top + W3a_bot)     U = W1 @ W3a_bot
    sm    = src @ M                               # (L, D)   fp8 DR
    score = (sm @ trg.T + row_n) / sqrt(O)        # (L, N)   fp8 DR
      (row_n = trg @ (W2 @ b1) + b1@b2; the per-l term sv@b2 is dropped -
       softmax over n is invariant to it)
    e     = exp(score)                            #          exp on ACT -> fp8
    tvp   = trg @ U                               # (N, O)   fp8 DR
    ctxr  = e @ tvp                               # (L, O)   fp8 DR, unnormalized
    sv3   = src @ V                               # (L, O)   bf16
    h2    = relu(sv3 - ctxr/sum_n(e) + r3)        # r3 = b3a + b1 @ W3a_top
    out   = h2 @ W3b + b3b                        # (L, O)   bf16

6 full-size matmul units instead of the reference's 8; the 4 attention-path
units run as fp8e4 DoubleRow (2 K-subtiles per instruction, ~2x bf16 rate).
fp8 is safe on this path: scores have std ~0.33 so softmax is near-uniform
and ctx has std ~0.008 vs h's 0.195 - quantization errors there are
attenuated ~25x. sv3/fc3b pass errors straight through, so they stay bf16.
The softmax denominator is summed off-PE (DVE slab adds + gpsimd partition
all-reduce) and folded into the DVE combine, so the PE runs nothing but the
2048 full-size matmuls. The PE dense region runs gap-free at the hardware
instruction floor (2048 x ~217ns); the last chunk's output stores are split
per-oc-half across the sync+scalar queues so the end-of-kernel drain is
~1.5us instead of ~4us. Measured ~470 us vs the 960 us all-bf16 8-unit
baseline; remaining non-PE time is fixed NEFF prologue/epilogue (~14us).

Sharding: data-parallel over B=32 across 8 cores (4 batch elems per core);
weights replicated. Host marshals activations to [D, tokens] layouts and
quantizes with fixed power-of-2 scales (inputs have std 1; >2.8x headroom
to the fp8e4 max of 240 everywhere).
"""

import math
from contextlib import ExitStack

import ml_dtypes
import numpy as np

import concourse.bass as bass
import concourse.mybir as mybir
import concourse.tile as tile
from concourse import bacc
from concourse.bass_isa import ReduceOp
from concourse.bass_utils import run_bass_kernel_spmd

P = 128
B_FULL = 32
N_CORES = 8
BS = B_FULL // N_CORES  # 4 batch elements per core
L = 1024
N = 1024
D = 1024
O = 1024

F32 = mybir.dt.float32
BF16 = mybir.dt.bfloat16
F8 = mybir.dt.float8e4
AF = mybir.ActivationFunctionType
ALU = mybir.AluOpType
DR = mybir.MatmulPerfMode.DoubleRow
NP_BF16 = ml_dtypes.bfloat16
NP_F8 = ml_dtypes.float8_e4m3

LC = 512          # l-chunk size for phase B
N_LC = L // LC    # 2 chunks
KD = D // P       # 8 contraction tiles over D
KO = O // P       # 8 tiles over O
KN = N // P       # 8 tiles over N
KH = KD // 2      # 4 DoubleRow steps over D

# fixed power-of-2 quantization scales (inputs are N(0,1); margins >= 2.8x)
S_SRC = 16.0
S_TRG = 16.0
S_M = 2048.0
S_SM = 32.0
S_E = 16.0
S_U = 4096.0
S_TVP = 64.0

CS_SM = S_SM / (S_SRC * S_M)                  # sm psum -> fp8 drain scale
CS_E = 1.0 / (S_SM * S_TRG * math.sqrt(O))    # score psum -> exp input scale
CS_TVP = S_TVP / (S_TRG * S_U)                # tvp psum -> fp8 drain scale
C_CTX = -1.0 / S_TVP                          # ctx*rbc -> true-value scale


def _load_weight(nc, dst, w_dram, ktiles):
    # one 3D-AP DMA: dram (ktiles*128, X) rows -> sbuf [128, ktiles, X]
    nc.sync.dma_start(
        dst[:, :, :], w_dram.ap().rearrange("(kk p) d -> p kk d", p=P))


def _load_st(nc, dest, dram, b):
    nc.sync.dma_start(
        dest[:, :, :], dram.ap()[b].rearrange("(kk p) d -> p kk d", p=P))


def _build(nc, tc):
    src8_d = nc.dram_tensor("srcT8", [BS, D, L], F8, kind="ExternalInput")
    trg8_d = nc.dram_tensor("trgT8", [BS, D, N], F8, kind="ExternalInput")
    srcbf_d = nc.dram_tensor("srcTbf", [BS, D, L], BF16, kind="ExternalInput")
    m8_d = nc.dram_tensor("M8", [D, D], F8, kind="ExternalInput")
    u8_d = nc.dram_tensor("U8", [D, O], F8, kind="ExternalInput")
    vbf_d = nc.dram_tensor("Vbf", [D, O], BF16, kind="ExternalInput")
    w3b_d = nc.dram_tensor("W3bbf", [O, O], BF16, kind="ExternalInput")
    r3_d = nc.dram_tensor("r3", [O], F32, kind="ExternalInput")
    b3b_d = nc.dram_tensor("b3bf32", [O], F32, kind="ExternalInput")
    ebias_d = nc.dram_tensor("ebias", [BS, N], F32, kind="ExternalInput")
    out = nc.dram_tensor("out", [BS, L, O], F32, kind="ExternalOutput")

    ctx = ExitStack()
    singles = ctx.enter_context(tc.tile_pool(name="singles", bufs=1))
    stp = ctx.enter_context(tc.tile_pool(name="stp", bufs=2))
    actp = ctx.enter_context(tc.tile_pool(name="actp", bufs=1))
    lcp = ctx.enter_context(tc.tile_pool(name="lcp", bufs=1))
    smallp = ctx.enter_context(tc.tile_pool(name="smallp", bufs=2))
    hlp = ctx.enter_context(tc.tile_pool(name="hlp", bufs=3))
    outp = ctx.enter_context(tc.tile_pool(name="outp", bufs=3))
    psum = ctx.enter_context(tc.tile_pool(name="psum", bufs=8, space="PSUM"))

    # ---- constants ----
    m8 = singles.tile([P, KD, D], F8)
    u8 = singles.tile([P, KD, O], F8)
    vbf = singles.tile([P, KD, O], BF16)
    w3b = singles.tile([P, KO, O], BF16)
    r3col = singles.tile([P, KO], F32)
    b3bfull = singles.tile([P, O], F32)
    # batch-0 activations + the weights needed first: the very first matmul
    # group (sm, j=0) reads only m8/s8 k-slabs 0-1, so those DMAs go first.
    s8_0 = stp.tile([P, KD, L], F8, tag="st")
    t8_0 = stp.tile([P, KD, N], F8, tag="st")
    sbf_0 = stp.tile([P, KD, L], BF16, tag="st")
    eb0 = smallp.tile([P, KN], F32, tag="ebias")
    # staged loads: the first sm matmul gates only on the first k-slab pair
    # of m8/s8; 3 DMAs/queue stay within the 4-deep queue-semaphore rotation
    def _stage(dst, dram_ap):
        for lo, hi in ((0, 2), (2, 4), (4, KD)):
            yield dst[:, lo:hi, :], dram_ap[lo * P:hi * P, :].rearrange(
                "(kk p) d -> p kk d", p=P)
    for dst_ap, src_ap in _stage(m8, m8_d.ap()):
        nc.sync.dma_start(dst_ap, src_ap)
    for dst_ap, src_ap in _stage(s8_0, src8_d.ap()[0]):
        nc.scalar.dma_start(dst_ap, src_ap)
    nc.sync.dma_start(
        u8[:, :, :], u8_d.ap().rearrange("(kk p) d -> p kk d", p=P))
    nc.scalar.dma_start(
        t8_0[:, :, :], trg8_d.ap()[0].rearrange("(kk p) d -> p kk d", p=P))
    nc.sync.dma_start(eb0[:], ebias_d.ap()[0].rearrange("(nn np) -> np nn", np=P))
    nc.sync.dma_start(r3col[:], r3_d.ap().rearrange("(oo op) -> op oo", op=P))
    nc.sync.dma_start(
        b3bfull[:], bass.AP(tensor=b3b_d.ap().tensor, offset=0, ap=[[0, P], [1, O]]))
    nc.scalar.dma_start(
        sbf_0[:, :, :], srcbf_d.ap()[0].rearrange("(kk p) d -> p kk d", p=P))
    _load_weight(nc, vbf, vbf_d, KD)
    _load_weight(nc, w3b, w3b_d, KO)

    for b in range(BS):
        if b == 0:
            s8, t8, sbf, ebcol = s8_0, t8_0, sbf_0, eb0
        else:
            s8 = stp.tile([P, KD, L], F8, tag="st")
            t8 = stp.tile([P, KD, N], F8, tag="st")
            sbf = stp.tile([P, KD, L], BF16, tag="st")
            ebcol = smallp.tile([P, KN], F32, tag="ebias")
            # prefetches ride the scalar hwdge queue (idle after batch 0) so
            # the sync queue carries only output stores - a 2MB prefetch on
            # the store queue would backpressure the 3-deep outp pool
            nc.scalar.dma_start(
                ebcol[:], ebias_d.ap()[b].rearrange("(nn np) -> np nn", np=P))
            for dram, dest in ((src8_d, s8), (trg8_d, t8), (srcbf_d, sbf)):
                nc.scalar.dma_start(
                    dest[:, :, :],
                    dram.ap()[b].rearrange("(kk p) d -> p kk d", p=P))

        # ---- phase A: sm = src@M and tvp = trg@U, both fp8 DoubleRow ----
        sm8 = actp.tile([P, KD, L], F8, tag="sm8")
        tvp8 = actp.tile([P, KN, O], F8, tag="tvp8")
        for j in range(KD):
            for lc in range(N_LC):
                ps = psum.tile([P, LC], F32)
                for kk in range(KH):
                    nc.tensor.matmul(
                        ps[:], m8[:, 2 * kk:2 * kk + 2, j * P:(j + 1) * P],
                        s8[:, 2 * kk:2 * kk + 2, lc * LC:(lc + 1) * LC],
                        start=(kk == 0), stop=(kk == KH - 1), perf_mode=DR)
                nc.scalar.activation(
                    sm8[:, j, lc * LC:(lc + 1) * LC], ps[:], AF.Identity,
                    scale=CS_SM)
        for i in range(KN):
            for oc in range(O // LC):
                ps = psum.tile([P, LC], F32)
                for kk in range(KH):
                    nc.tensor.matmul(
                        ps[:], t8[:, 2 * kk:2 * kk + 2, i * P:(i + 1) * P],
                        u8[:, 2 * kk:2 * kk + 2, oc * LC:(oc + 1) * LC],
                        start=(kk == 0), stop=(kk == KH - 1), perf_mode=DR)
                nc.scalar.activation(
                    tvp8[:, i, oc * LC:(oc + 1) * LC], ps[:], AF.Identity,
                    scale=CS_TVP)

        # ---- phase B: per l-chunk ----
        for lc in range(N_LC):
            lsl = slice(lc * LC, (lc + 1) * LC)
            # scoreT [n-part, l] fp8 DR; e8 = exp(score + row_n/sqrt(O)) * S_E
            e8 = lcp.tile([P, KN, LC], F8, tag="e8")
            for i in range(KN):
                ps = psum.tile([P, LC], F32)
                for kk in range(KH):
                    nc.tensor.matmul(
                        ps[:], t8[:, 2 * kk:2 * kk + 2, i * P:(i + 1) * P],
                        sm8[:, 2 * kk:2 * kk + 2, lsl],
                        start=(kk == 0), stop=(kk == KH - 1), perf_mode=DR)
                nc.scalar.activation(e8[:, i, :], ps[:], AF.Exp,
                                     scale=CS_E, bias=ebcol[:, i:i + 1])
            # sv3 (bf16) j=0..1 gives the PE work while ACT drains e8
            sv3s = lcp.tile([P, KO, LC], BF16, tag="sv3s")

            def sv3_group(j):
                ps = psum.tile([P, LC], F32)
                for k in range(KD):
                    nc.tensor.matmul(
                        ps[:], vbf[:, k, j * P:(j + 1) * P], sbf[:, k, lsl],
                        start=(k == 0), stop=(k == KD - 1))
                nc.scalar.activation(sv3s[:, j, :], ps[:], AF.Identity,
                                     bias=r3col[:, j:j + 1])

            for j in range(2):
                sv3_group(j)
            # denominator: DVE-sum the 8 n-tile slabs, gpsimd-reduce across
            # partitions (PE stays out of the softmax normalization entirely)
            e_sum = smallp.tile([P, LC], F32, tag="esum", bufs=1)
            nc.vector.tensor_add(e_sum[:], e8[:, 0, :], e8[:, 1, :])
            for i in range(2, KN):
                nc.vector.tensor_add(e_sum[:], e_sum[:], e8[:, i, :])
            d_bc = smallp.tile([P, LC], F32, tag="dbc", bufs=1)
            nc.gpsimd.partition_all_reduce(d_bc[:], e_sum[:], P, ReduceOp.add)
            for j in range(2, 4):
                sv3_group(j)
            rbc = smallp.tile([P, LC], F32, tag="rbc", bufs=1)
            nc.vector.reciprocal(rbc[:], d_bc[:])
            for j in range(4, KO):
                sv3_group(j)
            # ctx_raw (fp8 DR on unnormalized e8); normalization folds into
            # the DVE combine: h = sv3 - (ctx_raw*rbc)/S_TVP + r3
            h2 = lcp.tile([P, KO, LC], BF16, tag="h2")
            for j in range(KO):
                ps = psum.tile([P, LC], F32)
                for i in range(KH):
                    nc.tensor.matmul(
                        ps[:], tvp8[:, 2 * i:2 * i + 2, j * P:(j + 1) * P],
                        e8[:, 2 * i:2 * i + 2, :],
                        start=(i == 0), stop=(i == KH - 1), perf_mode=DR)
                ctxn = hlp.tile([P, LC], F32, tag="ctxn")
                nc.vector.tensor_mul(ctxn[:], ps[:], rbc[:])
                hl = hlp.tile([P, LC], BF16, tag="hl")
                nc.vector.scalar_tensor_tensor(
                    hl[:], ctxn[:], C_CTX, sv3s[:, j, :],
                    op0=ALU.mult, op1=ALU.add)
                nc.scalar.activation(h2[:, j, :], hl[:], AF.Relu)
            # fc3b: out natural [l-part, o] bf16, + b3b; one 1KB-row DMA per lt.
            # For the very last chunk, stores split per-oc (and the final oc
            # per-half) across the sync+scalar queues - both idle by then -
            # so the post-compute drain shrinks from ~4us to ~1.5us.
            last_chunk = (b == BS - 1 and lc == N_LC - 1)
            for lt in range(LC // P):
                o_sb = outp.tile([P, O], F32, tag="osb")
                row0 = lc * LC + lt * P
                for oc in range(O // LC):
                    ps = psum.tile([P, LC], F32)
                    for k in range(KO):
                        nc.tensor.matmul(
                            ps[:], h2[:, k, lt * P:(lt + 1) * P],
                            w3b[:, k, oc * LC:(oc + 1) * LC],
                            start=(k == 0), stop=(k == KO - 1))
                    final_oc = last_chunk and lt == LC // P - 1 and oc == O // LC - 1
                    if final_oc:
                        # split the critical last add+store into two halves
                        for hh in range(2):
                            sl = slice(oc * LC + hh * (LC // 2),
                                       oc * LC + (hh + 1) * (LC // 2))
                            psl = slice(hh * (LC // 2), (hh + 1) * (LC // 2))
                            nc.vector.tensor_add(o_sb[:, sl], ps[:, psl],
                                                 b3bfull[:, sl])
                            q = nc.sync if hh == 0 else nc.scalar
                            q.dma_start(
                                out.ap()[b, row0:row0 + P, sl], o_sb[:, sl])
                    else:
                        nc.vector.tensor_add(o_sb[:, oc * LC:(oc + 1) * LC],
                                             ps[:],
                                             b3bfull[:, oc * LC:(oc + 1) * LC])
                        if last_chunk:
                            q = nc.sync if (lt + oc) % 2 == 0 else nc.scalar
                            q.dma_start(
                                out.ap()[b, row0:row0 + P,
                                         oc * LC:(oc + 1) * LC],
                                o_sb[:, oc * LC:(oc + 1) * LC])
                if not last_chunk:
                    nc.sync.dma_start(out.ap()[b, row0:row0 + P, :], o_sb[:])

    ctx.close()


_NC_CACHE = None


def _get_nc():
    global _NC_CACHE
    if _NC_CACHE is None:
        nc = bacc.Bacc("TRN2", target_bir_lowering=False, debug=False,
                       num_devices=N_CORES)
        with tile.TileContext(nc) as tc:
            _build(nc, tc)
        nc.compile()
        _NC_CACHE = nc
    return _NC_CACHE


def _q8(x, scale):
    return np.clip(x * scale, -240.0, 240.0).astype(NP_F8)


def kernel(**inputs):
    nc = _get_nc()
    src = np.asarray(inputs["src"], dtype=np.float32)
    trg = np.asarray(inputs["trg"], dtype=np.float32)
    W1 = np.asarray(inputs["W1"], np.float32)
    W2 = np.asarray(inputs["W2"], np.float32)
    W3a = np.asarray(inputs["W3a"], np.float32)
    W3b = np.asarray(inputs["W3b"], np.float32)
    b1 = np.asarray(inputs["b1"], np.float32)
    b2 = np.asarray(inputs["b2"], np.float32)
    b3a = np.asarray(inputs["b3a"], np.float32)
    b3b = np.asarray(inputs["b3b"], np.float32)

    # host-side weight folding (O(D^3) once, vs O(B*L*D^2) on device)
    W3a_top, W3a_bot = W3a[:O], W3a[O:]
    M = W1 @ W2.T
    U = W1 @ W3a_bot
    V = W1 @ (W3a_top + W3a_bot)
    r3 = b3a + b1 @ W3a_top
    ebias = (trg @ (W2 @ b1) + float(b1 @ b2)) / math.sqrt(O) + math.log(S_E)

    src_t = np.ascontiguousarray(src.transpose(0, 2, 1))   # (B, D, L)
    trg_t = np.ascontiguousarray(trg.transpose(0, 2, 1))   # (B, D, N)
    shared = {
        "M8": _q8(M, S_M),
        "U8": _q8(U, S_U),
        "Vbf": V.astype(NP_BF16),
        "W3bbf": W3b.astype(NP_BF16),
        "r3": np.ascontiguousarray(r3),
        "b3bf32": np.ascontiguousarray(b3b),
    }
    src8 = _q8(src_t, S_SRC)
    trg8 = _q8(trg_t, S_TRG)
    srcbf = src_t.astype(NP_BF16)
    in_maps = []
    for c in range(N_CORES):
        m = dict(shared)
        m["srcT8"] = src8[c * BS:(c + 1) * BS]
        m["trgT8"] = trg8[c * BS:(c + 1) * BS]
        m["srcTbf"] = srcbf[c * BS:(c + 1) * BS]
        m["ebias"] = np.ascontiguousarray(ebias[c * BS:(c + 1) * BS])
        in_maps.append(m)
    res = run_bass_kernel_spmd(nc, in_maps, core_ids=list(range(N_CORES)))
    return np.concatenate([r["out"] for r in res.results], axis=0)

